# revision 39
# baseline (speedup 1.0000x reference)
"""SC-LSTM decoder (2-layer, teacher-forced) Trainium2 Bass kernel — v2.

Strategy (8 NeuronCores, tensor-parallel over H):
  - Core j owns gate columns [128j, 128j+128) of each layer's hidden state
    (GS=512 packed gate cols) and V-cols [256j, 256j+256) of the output
    projection.  Full batch B=128 on every core fills the PE stationary dim.
  - All GEMM inputs are bf16 (PSUM accumulates fp32).  Per gate block the
    x-contribution, h-contributions and (for r) the shared semantic-gate
    terms accumulate into a single PSUM group — the PE does the adds, so
    there is no separate x-precompute phase, no DRAM scratch, and no
    vector-engine gsum/rpre additions.
  - Two bf16 AllGathers per step rebuild the full transposed hidden state
    (32KB/rank each).  AG windows are filled with the output projection of
    the previous step, the next step's x-GEMMs, and the next step's
    precomputable layer-0 gate work.
"""

import sys

sys.path.insert(0, "/opt/trn_rl_repo")

import numpy as np

import concourse.bass as bass
import concourse.mybir as mybir
import concourse.tile as tile
from concourse import bacc
from concourse.bass_utils import run_bass_kernel_spmd
from concourse.masks import make_identity

B, T, E, H, D, V, L = 128, 100, 2048, 1024, 256, 2048, 2
NC = 8
P = 128
HS = H // NC      # 128 h-rows per core per layer
GS = 4 * HS       # 512 packed gate cols per core
VS = V // NC      # 256 output cols per core
KE = E // P       # 16 k-tiles over E
KH = H // P       # 8 k-tiles over H
DK = D // P       # 2 k-tiles over D
XB = 4            # x-tile DMA batch (steps per load)
F32 = mybir.dt.float32
BF16 = mybir.dt.bfloat16

_cache = {}


def _build(t_steps: int):
    nc = bacc.Bacc("TRN2", target_bir_lowering=False, debug=False, num_devices=NC)

    # ---------------- I/O declarations (per-core values supplied via in_maps)
    xT = nc.dram_tensor("xT", [E, t_steps * B], BF16, kind="ExternalInput")
    h0T_i = nc.dram_tensor("h0T_i", [H, B], BF16, kind="ExternalInput")
    c_i = nc.dram_tensor("c_i", [B, HS], F32, kind="ExternalInput")
    d_i = nc.dram_tensor("d_i", [B, D], F32, kind="ExternalInput")
    Wx0 = nc.dram_tensor("Wx0", [E, GS], BF16, kind="ExternalInput")
    Wx1x = nc.dram_tensor("Wx1x", [E, GS], BF16, kind="ExternalInput")
    Wh0 = nc.dram_tensor("Wh0", [H, GS], BF16, kind="ExternalInput")
    Wh1 = nc.dram_tensor("Wh1", [H, GS], BF16, kind="ExternalInput")
    Wx1h = nc.dram_tensor("Wx1h", [H, GS], BF16, kind="ExternalInput")
    Wrx01 = nc.dram_tensor("Wrx01", [E, 2 * D], BF16, kind="ExternalInput")
    Wrc0d = nc.dram_tensor("Wrc0d", [H, 2 * D], BF16, kind="ExternalInput")
    Wrc1d = nc.dram_tensor("Wrc1d", [H, 2 * D], BF16, kind="ExternalInput")
    Wr1h = nc.dram_tensor("Wr1h", [H, D], BF16, kind="ExternalInput")
    Wdc0 = nc.dram_tensor("Wdc0", [D, HS], BF16, kind="ExternalInput")
    Wdc1 = nc.dram_tensor("Wdc1", [D, HS], BF16, kind="ExternalInput")
    Wout = nc.dram_tensor("Wout", [2 * H, VS], BF16, kind="ExternalInput")

    out_o = nc.dram_tensor("out", [t_steps, B, VS], F32, kind="ExternalOutput")

    rg = [list(range(NC))]
    Sig = mybir.ActivationFunctionType.Sigmoid
    Tanh = mybir.ActivationFunctionType.Tanh
    mul = mybir.AluOpType.mult
    add = mybir.AluOpType.add

    with tile.TileContext(nc) as tc:
        with (
            tc.tile_pool(name="const", bufs=1) as constp,
            tc.tile_pool(name="wr", bufs=1) as wrp,
            tc.tile_pool(name="st", bufs=2) as stp,
            tc.tile_pool(name="xb", bufs=2) as xbp,
            tc.tile_pool(name="wk", bufs=2) as wkp,
            tc.tile_pool(name="psg", bufs=1, space="PSUM") as psg,
            tc.tile_pool(name="psr", bufs=1, space="PSUM") as psr,
            tc.tile_pool(name="pso", bufs=1, space="PSUM") as pso,
            tc.tile_pool(name="pst", bufs=2, space="PSUM") as pst,
            tc.tile_pool(name="dma_b", bufs=4, space="DRAM") as dramp,
        ):
            ident = constp.tile([P, P], F32)
            make_identity(nc, ident[:])

            # ---------------- resident weights
            wx0 = wrp.tile([P, KE, GS], BF16)
            wx1x = wrp.tile([P, KE, GS], BF16)
            wh0 = wrp.tile([P, KH, GS], BF16)
            wh1 = wrp.tile([P, KH, GS], BF16)
            wx1h = wrp.tile([P, KH, GS], BF16)
            wrx01 = wrp.tile([P, KE, 2 * D], BF16)
            wrc0d = wrp.tile([P, KH, 2 * D], BF16)
            wrc1d = wrp.tile([P, KH, 2 * D], BF16)
            wr1h = wrp.tile([P, KH, D], BF16)
            wdc0 = wrp.tile([P, DK, HS], BF16)
            wdc1 = wrp.tile([P, DK, HS], BF16)
            wout = wrp.tile([P, 2 * KH, VS], BF16)
            for dst, src in [
                (wx0, Wx0), (wx1x, Wx1x), (wh0, Wh0), (wh1, Wh1), (wx1h, Wx1h),
                (wrx01, Wrx01), (wrc0d, Wrc0d), (wrc1d, Wrc1d),
                (wr1h, Wr1h), (wdc0, Wdc0), (wdc1, Wdc1), (wout, Wout),
            ]:
                nc.sync.dma_start(dst[:], src.rearrange("(k p) n -> p k n", p=P))

            # ---------------- initial state
            h0T = stp.tile([P, KH, B], BF16, tag="h0T", name="h0Ti")
            h1T = stp.tile([P, KH, B], BF16, tag="h1T", name="h1Ti")
            nc.sync.dma_start(h0T[:], h0T_i.rearrange("(k p) n -> p k n", p=P))
            nc.sync.dma_start(h1T[:], h0T_i.rearrange("(k p) n -> p k n", p=P))
            c0 = stp.tile([B, HS], F32, tag="c0", name="c0i")
            c1 = stp.tile([B, HS], F32, tag="c1", name="c1i")
            nc.sync.dma_start(c0[:], c_i[:])
            nc.sync.dma_start(c1[:], c_i[:])
            d0 = stp.tile([B, D], F32, tag="d0", name="d0i")
            d1 = stp.tile([B, D], F32, tag="d1", name="d1i")
            nc.sync.dma_start(d0[:], d_i[:])
            nc.sync.dma_start(d1[:], d_i[:])

            # ---------------- x tiles (XB steps per DMA)
            def load_xbatch(u0):
                xtb = xbp.tile([P, KE, XB * B], BF16, tag="xtb", name=f"xtb{u0}")
                nb = min(XB, t_steps - u0) * B
                nc.scalar.dma_start(
                    xtb[:, :, :nb],
                    xT[:, u0 * B : u0 * B + nb].rearrange("(k p) n -> p k n", p=P),
                )
                return xtb

            xring = [None, None]
            xring[0] = load_xbatch(0)
            if t_steps > XB:
                xring[1] = load_xbatch(XB)

            def xk(t, k):
                """k-tile AP of x for step t."""
                xtb = xring[(t // XB) % 2]
                s = t % XB
                return xtb[:, k, s * B : (s + 1) * B]

            # ---------------- helpers
            def mm(ps, lhsT, rhs, st_, sp_):
                return nc.tensor.matmul(ps, lhsT, rhs, start=st_, stop=sp_)

            def x_parts(t, which, pin_after=None):
                """x-contribution k-tiles (start each PSUM group)."""
                if which == "g0":
                    ps = psg.tile([B, GS], F32, tag="g0p", bufs=1, name=f"g0p{t}")
                    w = wx0
                elif which == "g1":
                    ps = psg.tile([B, GS], F32, tag="g1p", bufs=2, name=f"g1p{t}")
                    w = wx1x
                else:
                    ps = psr.tile([B, 2 * D], F32, tag="rp", bufs=2, name=f"rp{t}")
                    w = wrx01
                for k in range(KE):
                    m = mm(ps[:], xk(t, k), w[:, k, :], k == 0, False)
                    if k == 0 and pin_after is not None:
                        bass._add_dep_helper(m.ins, pin_after.ins, sync=True,
                                             reason="pin filler into AG window")
                return ps

            def h_parts(ps, hT, w, stop, pin_after=None):
                for k in range(KH):
                    m = mm(ps[:], hT[:, k, :], w[:, k, :], False, stop and k == KH - 1)
                    if k == 0 and pin_after is not None:
                        bass._add_dep_helper(m.ins, pin_after.ins, sync=True,
                                             reason="pin filler into AG window")

            def gate_pre(gp, c_cur, li, t):
                """sigmoid/tanh of gate block + candidate-cell partial."""
                sig = wkp.tile([B, 3 * HS], F32, tag=f"sig{li}", name=f"sig{li}_{t}")
                nc.scalar.activation(sig[:], gp[:, : 3 * HS], Sig)
                tgc = wkp.tile([B, HS], F32, tag=f"tgc{li}", name=f"tgc{li}_{t}")
                nc.scalar.activation(tgc[:], gp[:, 3 * HS :], Tanh)
                t1 = wkp.tile([B, HS], F32, tag=f"t1{li}", name=f"t1{li}_{t}")
                nc.vector.tensor_tensor(t1[:], sig[:, :HS], tgc[:], mul)
                m2 = wkp.tile([B, HS], F32, tag=f"m2{li}", name=f"m2{li}_{t}")
                nc.vector.tensor_tensor(m2[:], sig[:, HS : 2 * HS], c_cur[:], mul)
                cpart = wkp.tile([B, HS], F32, tag=f"cp{li}", name=f"cp{li}_{t}")
                nc.vector.tensor_tensor(cpart[:], t1[:], m2[:], add)
                return sig, cpart

            def r_dc_path(rp_ap, d_cur, wdc, li, t):
                sr = wkp.tile([B, D], F32, tag=f"sr{li}", name=f"sr{li}_{t}")
                nc.scalar.activation(sr[:], rp_ap, Sig)
                d_new = stp.tile([B, D], F32, tag=f"d{li}", name=f"d{li}_{t}")
                nc.vector.tensor_tensor(d_new[:], sr[:], d_cur[:], mul)
                # scr bank layout: [0:256) dt-transpose, [256:384) dc matmul,
                # [384:512) nh-transpose
                scr = pst.tile([P, 4 * B], F32, tag="scr", name=f"scr{li}_{t}")
                for k in range(DK):
                    nc.tensor.transpose(scr[:, k * B : (k + 1) * B],
                                        d_new[:, k * P : (k + 1) * P], ident[:])
                dtT = wkp.tile([P, DK * B], BF16, tag=f"dtT{li}", name=f"dtT{li}_{t}")
                nc.vector.tensor_copy(dtT[:], scr[:, : DK * B])
                for k in range(DK):
                    mm(scr[:, 2 * B : 3 * B], dtT[:, k * B : (k + 1) * B],
                       wdc[:, k, :], k == 0, k == DK - 1)
                tdc = wkp.tile([B, HS], F32, tag=f"tdc{li}", name=f"tdc{li}_{t}")
                nc.scalar.activation(tdc[:], scr[:, 2 * B : 3 * B], Tanh)
                return tdc, d_new, scr

            def finish_cell(cpart, tdc, sig, li, t):
                c_new = stp.tile([B, HS], F32, tag=f"c{li}", name=f"c{li}_{t}")
                nc.vector.tensor_tensor(c_new[:], cpart[:], tdc[:], add)
                th = wkp.tile([B, HS], F32, tag=f"th{li}", name=f"th{li}_{t}")
                nc.scalar.activation(th[:], c_new[:], Tanh)
                nh = wkp.tile([B, HS], F32, tag=f"nh{li}", name=f"nh{li}_{t}")
                nc.vector.tensor_tensor(nh[:], sig[:, 2 * HS :], th[:], mul)
                return nh, c_new

            def trigger_gather(nh, scr, li, t):
                ntr = nc.tensor.transpose(scr[:, 3 * B :], nh[:], ident[:])
                nhT = wkp.tile([P, B], BF16, tag=f"nhT{li}", name=f"nhT{li}_{t}")
                nc.vector.tensor_copy(nhT[:], scr[:, 3 * B :])
                agi = dramp.tile([P, B], BF16, tag=f"agi{li}", name=f"agi{li}_{t}")
                ago = dramp.tile([H, B], BF16, tag=f"ago{li}", addr_space="Shared",
                                 name=f"ago{li}_{t}")
                nc.sync.dma_start(agi[:], nhT[:])
                cc = nc.gpsimd.collective_compute(
                    "AllGather", mybir.AluOpType.bypass, replica_groups=rg,
                    ins=[agi[:]], outs=[ago[:]],
                )
                return ago, ntr

            def load_gathered(ago, li, t):
                hT = stp.tile([P, KH, B], BF16, tag=f"h{li}T", name=f"h{li}T_{t}")
                nc.sync.dma_start(
                    hT[:, : KH // 2, :],
                    ago[: H // 2, :].rearrange("(k p) n -> p k n", p=P))
                nc.sync.dma_start(
                    hT[:, KH // 2 :, :],
                    ago[H // 2 :, :].rearrange("(k p) n -> p k n", p=P))
                return hT

            def out_proj(h0T_, h1T_, t):
                op = pso.tile([B, VS], F32, tag="outp", name=f"outp{t}")
                m0 = None
                for k in range(2 * KH):
                    src = h0T_[:, k, :] if k < KH else h1T_[:, k - KH, :]
                    m = mm(op[:], src, wout[:, k, :], k == 0, k == 2 * KH - 1)
                    if m0 is None:
                        m0 = m
                osb = wkp.tile([B, VS], F32, tag="osb", name=f"osb{t}")
                nc.vector.tensor_copy(osb[:], op[:])
                nc.scalar.dma_start(out_o[t], osb[:])
                return m0

            # ---------------- prologue: step-0 groups (x + h0 parts) + layer-0
            # gate-side precompute
            g0p = x_parts(0, "g0")
            rp = x_parts(0, "r")
            g1p = x_parts(0, "g1")
            h_parts(g0p, h0T, wh0, stop=True)
            h_parts(rp, h0T, wrc0d, stop=False)
            sig0, cpart0 = gate_pre(g0p, c0, 0, -1)

            def pin(inst, after):
                bass._add_dep_helper(inst.ins, after.ins, sync=True,
                                     reason="pin filler into AG window")

            # ---------------- main loop
            for t in range(t_steps):
                last = t == t_steps - 1
                # -- A: packed rc h1-part first (h1T = h1(t-1)), then the full
                #       layer-0 critical chain so its notify lands promptly
                h_parts(rp, h1T, wrc1d, stop=True)
                tdc0, d0, scr0 = r_dc_path(rp[:, :D], d0, wdc0, 0, t)
                nh0, c0 = finish_cell(cpart0, tdc0, sig0, 0, t)
                ago0, ntr0 = trigger_gather(nh0, scr0, 0, t)

                # -- B: remaining h1-dependent parts, pushed into the AG1
                #       window so they don't contend with the layer-0 chain
                h_parts(g1p, h1T, wh1, stop=False, pin_after=ntr0)

                # -- D: AG1 window fillers, pinned behind the nh0 transpose so
                #       they cannot precede it in the PE FIFO
                if t > 0:
                    m0 = out_proj(h0T, h1T, t - 1)
                    pin(m0, ntr0)
                if t % XB == 0 and t > 0 and t + XB < t_steps:
                    xring[(t // XB + 1) % 2] = load_xbatch(t + XB)
                if not last:
                    g0p_n = x_parts(t + 1, "g0", pin_after=ntr0)
                    rp_n = x_parts(t + 1, "r", pin_after=ntr0)

                # -- E/F: consume AG1 -> r1 nh0-part, then layer-1 r-path,
                #         then g1 nh0-part, then gates/cell
                h0T_new = load_gathered(ago0, 0, t)
                for k in range(KH):
                    nc.tensor.matmul(rp[:, D:], h0T_new[:, k, :], wr1h[:, k, :],
                                     start=False, stop=k == KH - 1,
                                     skip_group_check=True)
                tdc1, d1, scr1 = r_dc_path(rp[:, D:], d1, wdc1, 1, t)
                h_parts(g1p, h0T_new, wx1h, stop=True)
                sig1, cpart1 = gate_pre(g1p, c1, 1, t)
                nh1, c1 = finish_cell(cpart1, tdc1, sig1, 1, t)
                ago1, ntr1 = trigger_gather(nh1, scr1, 1, t)

                # -- H: AG2 window fillers (next step's x/g0/r h0-parts +
                #       layer-0 gate-side precompute), pinned after nh1 transpose
                if not last:
                    g1p_n = x_parts(t + 1, "g1", pin_after=ntr1)
                    h_parts(g0p_n, h0T_new, wh0, stop=True)
                    h_parts(rp_n, h0T_new, wrc0d, stop=False)
                    sig0, cpart0 = gate_pre(g0p_n, c0, 0, t)

                # -- I: consume AG2
                h1T_new = load_gathered(ago1, 1, t)

                h0T, h1T = h0T_new, h1T_new
                if not last:
                    g0p, g1p, rp = g0p_n, g1p_n, rp_n

            out_proj(h0T, h1T, t_steps - 1)

    nc.compile()
    return nc


def _prep_inputs(input_seq, h0, dt0, w2h_W0, w2h_b0, w2h_W1, w2h_b1,
                 w2hr_W0, w2hr_b0, w2hr_W1, w2hr_b1,
                 h2h_W0, h2h_b0, h2h_W1, h2h_b1,
                 h2hr_W0, h2hr_b0, h2hr_W1, h2hr_b1,
                 dc_W0, dc_W1, out_W, out_b, t_steps):
    f = np.float32
    bf = np.dtype("bfloat16") if hasattr(np, "bfloat16") else None
    import ml_dtypes
    bf = ml_dtypes.bfloat16
    for name, b in [("w2h_b0", w2h_b0), ("w2h_b1", w2h_b1), ("w2hr_b0", w2hr_b0),
                    ("w2hr_b1", w2hr_b1), ("h2h_b0", h2h_b0), ("h2h_b1", h2h_b1),
                    ("h2hr_b0", h2hr_b0), ("h2hr_b1", h2hr_b1), ("out_b", out_b)]:
        assert not np.any(np.asarray(b)), f"nonzero bias {name} unsupported"

    # time-step inputs: SOS one-hot at t=0, then input_seq[:, t-1]
    xs = np.empty((t_steps, B, E), f)
    xs[0] = 0.0
    xs[0, :, 0] = 1.0
    xs[1:] = np.asarray(input_seq, f).transpose(1, 0, 2)[: t_steps - 1]
    xT = np.ascontiguousarray(xs.reshape(t_steps * B, E).T).astype(bf)

    h0 = np.asarray(h0, f)
    h0T = np.ascontiguousarray(h0.T).astype(bf)
    dt0 = np.asarray(dt0, f)

    alpha = 1.0 / L
    cvt = lambda a: np.ascontiguousarray(np.asarray(a, f)).astype(bf)

    in_maps = []
    for j in range(NC):
        gc = np.r_[tuple(np.arange(g * H + j * HS, g * H + (j + 1) * HS) for g in range(4))]
        vs = slice(j * VS, (j + 1) * VS)
        in_maps.append({
            "xT": xT,
            "h0T_i": h0T,
            "c_i": np.ascontiguousarray(h0[:, j * HS : (j + 1) * HS]),
            "d_i": dt0,
            "Wx0": cvt(np.asarray(w2h_W0, f)[:, gc]),
            "Wx1x": cvt(np.asarray(w2h_W1, f)[:E, gc]),
            "Wh0": cvt(np.asarray(h2h_W0, f)[:, gc]),
            "Wh1": cvt(np.asarray(h2h_W1, f)[:, gc]),
            "Wx1h": cvt(np.asarray(w2h_W1, f)[E:, gc]),
            "Wrx01": cvt(np.concatenate(
                [np.asarray(w2hr_W0, f), np.asarray(w2hr_W1, f)[:E]], axis=1)),
            "Wrc0d": cvt(np.tile(np.asarray(h2hr_W0, f) * alpha, (1, 2))),
            "Wrc1d": cvt(np.tile(np.asarray(h2hr_W1, f) * alpha, (1, 2))),
            "Wr1h": cvt(np.asarray(w2hr_W1, f)[E:]),
            "Wdc0": cvt(np.asarray(dc_W0, f)[:, j * HS : (j + 1) * HS]),
            "Wdc1": cvt(np.asarray(dc_W1, f)[:, j * HS : (j + 1) * HS]),
            "Wout": cvt(np.asarray(out_W, f)[:, vs]),
        })
    return in_maps


def _run(t_steps, trace, **inputs):
    if trace:
        import prof_shim

        prof_shim.install()
    key = t_steps
    if key not in _cache:
        _cache[key] = _build(t_steps)
    nc = _cache[key]
    in_maps = _prep_inputs(**inputs, t_steps=t_steps)
    res = run_bass_kernel_spmd(nc, in_maps, list(range(NC)), trace=trace)
    parts = [res.results[j]["out"] for j in range(NC)]  # each (T, B, VS)
    full = np.concatenate(parts, axis=2)                # (T, B, V)
    return np.ascontiguousarray(full.transpose(1, 0, 2)), res


def kernel(**inputs) -> np.ndarray:
    out, _ = _run(T, False, **inputs)
    return out


def kernel_traced(t_steps=T, **inputs):
    out, res = _run(t_steps, True, **inputs)
    return out, res
